# revision 13
# baseline (speedup 1.0000x reference)
"""Trainium2 Bass kernel for nn_EnhanceSelfAttention (B=16, N=577, C=768, H=12).

Self-contained: takes full unsharded inputs, shards batch across 8 NeuronCores
(2 batches/core), runs a fused attention kernel per core, gathers the output.

v2.1: host-side data staging removes all on-device layout work:
  - x is transposed + cast to f16 on host -> xT [128, 6*1154]
  - weights repacked per 128-row chunk so every load is one contiguous DMA
  - relative-position bias + causal mask exp()'d on host, shipped as
    per-k-tile f16 tables in the exact SBUF layout
  - softmax scale folded into the q weights/bias on host
Schedule: batch-outer attention so the other batch's v-projection and the
first batch's output projection act as PE gap-fillers inside the
ScalarE/DVE-saturated attention windows.

Per-core pipeline (f16 matmul operands, fp32 PSUM):
  B. qT,kT = Wqk^T @ xT per head-pair (interleaved into D as filler)
  C. v = x @ Wv + b per k-tile in [k, 12*65] f16 (ones col -> denominator)
  D. per (batch, head-pair): sT = kT.T@qT (two heads via PE row groups),
     p = exp(sT) * expb, OT += v.T@p over causal k-tiles; row 64 of OT is
     the denominator; divide via DVE reciprocal + GpSimd broadcast.
  E. y = OT.T @ Wout + b, per-m-tile DMAs to DRAM.
"""

import numpy as np

import concourse.bass as bass
import concourse.tile as tile
from concourse import bacc, mybir
from concourse.bass_utils import run_bass_kernel_spmd

F32 = mybir.dt.float32
F16 = mybir.dt.float16

B, NTOK, CDIM, NH, DH = 16, 577, 768, 12, 64
GRID = 24
NRD = (2 * GRID - 1) * (2 * GRID - 1) + 3  # 2212
NCORES = 8
BLOC = B // NCORES       # batches per core
NSEQ = BLOC * NTOK       # 1154
SCALE = DH ** -0.5       # 0.125
NEG = -65504.0

QBLOCKS = [(0, 128), (128, 449)]            # (qstart, qN)
KTILES = [(0, 128), (128, 128), (256, 128), (384, 128), (512, 65)]
QLO = [k0 for k0, _ in KTILES]              # per-tile stored q range [QLO[t], 577)
WID = [NTOK - q for q in QLO]               # 577, 449, 321, 193, 65
NBLK = [(0, 386), (386, 384), (770, 384)]   # token blocks for B projections
# x DMA arrives in these column blocks; jp0's B units match the finer start
XBLK = [(0, 192), (192, 194), (386, 384), (770, 384)]
NBLK0 = [(0, 192), (192, 194), (386, 384), (770, 384)]
MCH = [(0, 289), (289, 288)]                # m-chunks within a batch for E

EXB_OFF = []
_off = 0
for _t, (_k0, _pw) in enumerate(KTILES):
    EXB_OFF.append(_off)
    _off += _pw * WID[_t] * NH
EXB_TOTAL = _off

_CACHE = {}


def _check_rel_index(ri):
    """Assert the expected structure of rel_index (sanity only)."""
    assert ri.shape == (NTOK, NTOK)
    assert ri[0, 0] == NRD - 1
    assert (ri[0, 1:] == NRD - 3).all()
    assert (ri[1:, 0] == NRD - 2).all()


def _host_prep(x, qkv_w, qkv_b, pos_emb, out_w, out_b, ri):
    """Build all per-core DMA images on host."""
    _check_rel_index(ri)
    f16 = np.float16

    # exp(bias + causal mask) tables, one per k-tile, in SBUF layout
    # expb[t][p, h*W + (q - qlo)] = exp(pos_emb[h, ri[q, k0+p]] + mask(q, k0+p))
    bias = pos_emb[:, ri]                                   # [12, 577, 577]
    mask = np.where(np.tri(NTOK, dtype=np.float32) > 0, 0.0, NEG)
    exb_full = np.exp(bias + mask[None]).astype(f16)        # [12, q, k]
    exb = np.empty(EXB_TOTAL, dtype=f16)
    for t, (k0, pw) in enumerate(KTILES):
        qlo = QLO[t]
        blk = exb_full[:, qlo:, k0:k0 + pw]                 # [12, W, pw]
        exb[EXB_OFF[t]:EXB_OFF[t] + pw * WID[t] * NH] = (
            np.ascontiguousarray(blk.transpose(2, 0, 1)).reshape(-1))

    # weights: per-chunk contiguous images
    wq = (qkv_w[:, :CDIM] * SCALE).astype(f16)              # scale folded in
    wk = qkv_w[:, CDIM:2 * CDIM].astype(f16)
    wv = qkv_w[:, 2 * CDIM:].astype(f16)
    wo = out_w.astype(f16)

    def chunked(w):  # [768, 768] -> [128, 6*768]: out[p, c*768+d] = w[c*128+p, d]
        return np.ascontiguousarray(
            w.reshape(6, 128, CDIM).transpose(1, 0, 2)).reshape(128, 6 * CDIM)

    # hqk[jp] = [128, 1536]: cols 0:768 q-chunk jp, cols 768:1536 k-chunk jp
    hqk = np.empty((6, 128, 2 * CDIM), dtype=f16)
    for jp in range(6):
        for part, w in ((0, wq), (1, wk)):
            blk = w[:, jp * 128:(jp + 1) * 128]             # [768, 128]
            hqk[jp, :, part * CDIM:(part + 1) * CDIM] = (
                blk.reshape(6, 128, 128).transpose(1, 0, 2).reshape(128, CDIM))
    hqk = np.ascontiguousarray(hqk.reshape(6 * 128, 2 * CDIM))
    hv = chunked(wv)
    hwo = chunked(wo)

    # bias columns for q/k activations: [128, 12] f32
    hbc = np.empty((128, 12), dtype=np.float32)
    for r in range(12):
        if r < 6:
            hbc[:, r] = qkv_b[r * 128:(r + 1) * 128] * SCALE
        else:
            hbc[:, r] = qkv_b[CDIM + (r - 6) * 128:CDIM + (r - 5) * 128]
    # bias row for v: [1, 768] f16; out-bias as columns [128, 6] f32
    hbrow = qkv_b[2 * CDIM:].astype(f16)[None, :]
    hocol = np.ascontiguousarray(out_b.reshape(6, 128).T.astype(np.float32))

    # per-core x images: [128, 6*1154], hx[p, c*1154+t] = xshard[t, c*128+p]
    hx = []
    for c in range(NCORES):
        shard = x[c * BLOC:(c + 1) * BLOC].reshape(NSEQ, CDIM)
        hx.append(np.ascontiguousarray(
            shard.T.astype(f16).reshape(6, 128, NSEQ).transpose(1, 0, 2)
        ).reshape(128, 6 * NSEQ))
    return exb, hqk, hv, hwo, hbc, hbrow, hocol, hx


def _build():
    nc = bacc.Bacc("TRN2", target_bir_lowering=False, debug=False)

    hx_d = nc.dram_tensor("hx", [128, 6 * NSEQ], F16, kind="ExternalInput").ap()
    hqk_d = nc.dram_tensor("hqk", [6 * 128, 2 * CDIM], F16,
                           kind="ExternalInput").ap()
    hv_d = nc.dram_tensor("hv", [128, 6 * CDIM], F16, kind="ExternalInput").ap()
    hwo_d = nc.dram_tensor("hwo", [128, 6 * CDIM], F16, kind="ExternalInput").ap()
    hbc_d = nc.dram_tensor("hbc", [128, 12], F32, kind="ExternalInput").ap()
    hbrow_d = nc.dram_tensor("hbrow", [1, CDIM], F16,
                             kind="ExternalInput").ap()
    hocol_d = nc.dram_tensor("hocol", [128, 6], F32, kind="ExternalInput").ap()
    exb_d = nc.dram_tensor("exb", [EXB_TOTAL], F16, kind="ExternalInput").ap()
    # output transposed: yT[c, t]; host transposes back (free)
    y_d = nc.dram_tensor("y", [CDIM, NSEQ], F16, kind="ExternalOutput").ap()

    with tile.TileContext(nc) as tc:
        _emit(nc, tc, hx_d, hqk_d, hv_d, hwo_d, hbc_d, hbrow_d, hocol_d,
              exb_d, y_d)
    nc.compile()
    return nc


def _emit(nc, tc, hx_d, hqk_d, hv_d, hwo_d, hbc_d, hbrow_d, hocol_d,
          exb_d, y_d):
    from contextlib import ExitStack

    with ExitStack() as top:
        persist = top.enter_context(tc.tile_pool(name="persist", bufs=1))
        consts = top.enter_context(tc.tile_pool(name="consts", bufs=1))

        # ---- persistent tiles ----
        xT = consts.tile([128, 6 * NSEQ], F16, tag="xT", name="xT")
        wqk = [consts.tile([128, 2 * CDIM], F16, tag=f"wqk{j}", name=f"wqk{j}")
               for j in range(6)]
        qkb = consts.tile([128, 12], F32, tag="qkb", name="qkb")
        brow = consts.tile([1, CDIM], F16, tag="brow", name="brow")
        ocol = consts.tile([128, 6], F32, tag="ocol", name="ocol")
        ones128 = consts.tile([1, 128], F16, tag="ones128", name="ones128")
        wvall = consts.tile([128, 6 * CDIM], F16, tag="wvall", name="wvall")
        woall = consts.tile([128, 6 * CDIM], F16, tag="woall", name="woall")

        qT = [persist.tile([128, NSEQ], F16, tag=f"qT{j}", name=f"qT{j}")
              for j in range(6)]
        kT = [persist.tile([128, NSEQ], F16, tag=f"kT{j}", name=f"kT{j}")
              for j in range(6)]
        vt = [[persist.tile([128, NH * 65], F16, tag=f"v{b}_{t}", name=f"v{b}_{t}")
               for t in range(5)] for b in range(BLOC)]
        oT = [persist.tile([128, NSEQ], F16, tag=f"oT{j}", name=f"oT{j}")
              for j in range(6)]
        expb = [persist.tile([128, WID[t] * NH], F16, tag=f"expb{t}",
                             name=f"expb{t}") for t in range(5)]
        vbias = consts.tile([128, CDIM], F32, tag="vbias", name="vbias")

        # ---- DMA issue: x on sync(SP); weights on scalar(Act) queue;
        # wv + exp-bias tables on gpsimd(Pool) queue ----
        xT3 = xT[:].rearrange("p (c t) -> p c t", c=6)

        def load_exb(t):
            k0, pw = KTILES[t]
            src = bass.AP(exb_d.tensor, EXB_OFF[t],
                          [[WID[t] * NH, pw], [1, WID[t] * NH]])
            nc.gpsimd.dma_start(expb[t][0:pw, :], src)

        def load_x_cols(t0, tw):
            nc.sync.dma_start(
                xT3[:, :, t0:t0 + tw],
                bass.AP(hx_d.tensor, t0, [[6 * NSEQ, 128], [NSEQ, 6], [1, tw]]))

        nc.scalar.dma_start(qkb[:], hbc_d[:, :])
        nc.scalar.dma_start(brow[:], hbrow_d[:, :])
        nc.scalar.dma_start(wqk[0][:], hqk_d[0:128, :])
        for t0, tw in XBLK:
            load_x_cols(t0, tw)
        for j in range(1, 6):
            nc.scalar.dma_start(wqk[j][:], hqk_d[j * 128:(j + 1) * 128, :])
        nc.gpsimd.dma_start(wvall[:, 0:3 * CDIM], hv_d[:, 0:3 * CDIM])
        nc.gpsimd.dma_start(wvall[:, 3 * CDIM:], hv_d[:, 3 * CDIM:])
        load_exb(0)
        load_exb(1)
        load_exb(2)
        load_exb(3)
        load_exb(4)
        nc.scalar.dma_start(ocol[:], hocol_d[:, :])
        nc.scalar.dma_start(woall[:, 0:3 * CDIM], hwo_d[:, 0:3 * CDIM])
        nc.scalar.dma_start(woall[:, 3 * CDIM:], hwo_d[:, 3 * CDIM:])

        nc.vector.memset(ones128[:], 1.0)

        # PE warm-up chain (ramps the clock while DMAs land) + vbias
        # broadcast to a [128, 768] tile
        with tc.tile_pool(name="bb_psum", bufs=2, space="PSUM") as bbps:
            wps = bbps.tile([1, 456], F32, tag="warm", name="warm")
            for _ in range(10):
                nc.tensor.matmul(wps[0:1, 0:128], ones128[0:1, 0:1],
                                 ones128[0:1, :], start=True, stop=True)
            for h0 in (0, 384):
                ps = bbps.tile([128, 384], F32, tag="bb", name="bb")
                nc.tensor.matmul(ps[:], ones128[:],
                                 brow[0:1, h0:h0 + 384],
                                 start=True, stop=True)
                nc.vector.tensor_copy(vbias[:, h0:h0 + 384], ps[:])

        # ---------------- emission helpers ----------------
        def emit_b(jp, r, nb0, nbw, ps_qk):
            part = 0 if r < 6 else 1
            dst = qT[jp] if r < 6 else kT[jp]
            ps = ps_qk.tile([128, 386], F32, tag="psqk", name="psqk")
            for c in range(6):
                nc.tensor.matmul(
                    ps[0:128, 0:nbw],
                    wqk[jp][:, part * CDIM + c * 128:part * CDIM + (c + 1) * 128],
                    xT3[:, c, nb0:nb0 + nbw],
                    start=(c == 0), stop=(c == 5))
            nc.scalar.activation(
                dst[:, nb0:nb0 + nbw], ps[0:128, 0:nbw],
                mybir.ActivationFunctionType.Identity,
                bias=qkb[:, r:r + 1])

        def b_units(jp, ps_qk):
            """Emission units for head-pair jp's q/k projections (jp0 uses
            finer first blocks so PE can start as soon as x cols 0:192 land)."""
            for nb0, nbw in (NBLK0 if jp == 0 else NBLK):
                for r in (jp, jp + 6):
                    def unit(r=r, nb0=nb0, nbw=nbw):
                        emit_b(jp, r, nb0, nbw, ps_qk)
                    yield unit

        def c_units(b, ps_v, eng):
            """5 emission units (one per k-tile) of the v-projection for
            batch b. eng: engine for the PSUM->SBUF bias add."""
            for t, (k0, pw) in enumerate(KTILES):
                def unit(t=t, k0=k0, pw=pw):
                    vtile = vt[b][t]
                    for half in range(2):
                        ps = ps_v.tile([128, 386], F32, tag="psqk",
                                       name="psv")
                        for c in range(6):
                            nc.tensor.matmul(
                                ps[0:pw, 0:384],
                                xT3[:, c, b * NTOK + k0:b * NTOK + k0 + pw],
                                wvall[:, c * CDIM + half * 384:
                                      c * CDIM + (half + 1) * 384],
                                start=(c == 0), stop=(c == 5))
                        dst = vtile[0:pw, :].rearrange(
                            "p (h d) -> p h d",
                            h=NH)[:, half * 6:(half + 1) * 6, 0:64]
                        src = ps[0:pw, 0:384].rearrange("p (h d) -> p h d",
                                                        d=64)
                        bsl = vbias[0:pw, half * 384:(half + 1) * 384].rearrange(
                            "p (h d) -> p h d", d=64)
                        eng.tensor_tensor(out=dst, in0=src, in1=bsl,
                                          op=mybir.AluOpType.add)
                    nc.vector.memset(
                        vtile[0:pw, :].rearrange("p (h d) -> p h d",
                                                 h=NH)[:, :, 64:65], 1.0)
                yield unit

        def e_units(b, ps_o, yt_pool):
            """12 emission units (co-chunk j x m-chunk) of the transposed
            output projection yT[j*128:(j+1)*128, b*577+m0:m0+mw] for batch
            b. out-bias is a per-partition column folded into the ScalarE
            PSUM->SBUF copy; y ships f16 and the host transposes."""
            for j in range(6):
                for mi, (m0, mw) in enumerate(MCH):
                    def unit(j=j, mi=mi, m0=m0, mw=mw):
                        ps = ps_o.tile([128, 289], F32, tag="pso", name="pso")
                        for c in range(6):
                            nc.tensor.matmul(
                                ps[0:128, 0:mw],
                                woall[:, c * CDIM + j * 128:
                                      c * CDIM + (j + 1) * 128],
                                oT[c][:, b * NTOK + m0:b * NTOK + m0 + mw],
                                start=(c == 0), stop=(c == 5))
                        yt = yt_pool.tile([128, 289], F16, tag="yt",
                                          name="yt")
                        nc.scalar.activation(
                            yt[0:128, 0:mw], ps[0:128, 0:mw],
                            mybir.ActivationFunctionType.Identity,
                            bias=ocol[:, j:j + 1])
                        q = nc.gpsimd if (j + mi) % 2 else nc.sync
                        q.dma_start(
                            bass.AP(y_d.tensor,
                                    j * 128 * NSEQ + b * NTOK + m0,
                                    [[NSEQ, 128], [1, mw]]),
                            yt[0:128, 0:mw])
                    yield unit

        with tc.tile_pool(name="yt_pool", bufs=4) as yt_pool, \
             tc.tile_pool(name="ps_sT", bufs=4, space="PSUM") as ps_sT, \
             tc.tile_pool(name="ps_OT", bufs=2, space="PSUM") as ps_OT, \
             tc.tile_pool(name="att_tmp", bufs=4) as att_tmp, \
             tc.tile_pool(name="es_pool", bufs=7) as es_pool, \
             tc.tile_pool(name="p_pool", bufs=7) as p_pool:

            def attn(jp, b, qstart, qN, psO, ocol, fill,
                     pool_mult=False):
                """Emit S/exp/mult/AV pipeline for one (jp, b, q-block).
                Returns the deferred normalization closure."""
                qend = qstart + qN
                valid_t = [t for t in range(5) if QLO[t] < qend]
                tlast = valid_t[-1]
                prev = None

                pt = {}

                def av(t):
                    k0, pw = KTILES[t]
                    qlo = max(qstart, QLO[t])
                    off = ocol + qlo - qstart
                    Nt = qend - qlo
                    for side in range(2):
                        h = 2 * jp + side
                        nc.tensor.matmul(
                            psO[side][0:65, off:off + Nt],
                            vt[b][t][0:pw, h * 65:(h + 1) * 65],
                            pt[t][side][0:pw, 0:Nt],
                            start=(t == valid_t[0]),
                            stop=(t == tlast))

                for i, t in enumerate(valid_t):
                    k0, pw = KTILES[t]
                    qlo = max(qstart, QLO[t])
                    Nt = qend - qlo
                    ebase = qlo - QLO[t]
                    psS = [ps_sT.tile([128, 456], F32, tag="psS",
                                      name="psS") for _ in range(2)]
                    for side in range(2):
                        r0 = side * 64
                        nc.tensor.matmul(
                            psS[side][0:pw, 0:Nt],
                            kT[jp][r0:r0 + 64,
                                   b * NTOK + k0:b * NTOK + k0 + pw],
                            qT[jp][r0:r0 + 64,
                                   b * NTOK + qlo:b * NTOK + qlo + Nt],
                            start=True, stop=True,
                            tile_position=(r0, 0))
                    pt[t] = []
                    for side in range(2):
                        h = 2 * jp + side
                        es = es_pool.tile([128, 456], F16, tag="es",
                                          name="es")
                        nc.scalar.activation(
                            es[0:pw, 0:Nt], psS[side][0:pw, 0:Nt],
                            mybir.ActivationFunctionType.Exp)
                        p = p_pool.tile([128, 456], F16, tag="p", name="p")
                        meng = nc.gpsimd if (pool_mult and side == 1) \
                            else nc.vector
                        meng.tensor_tensor(
                            out=p[0:pw, 0:Nt],
                            in0=es[0:pw, 0:Nt],
                            in1=expb[t][0:pw,
                                        h * WID[t] + ebase:
                                        h * WID[t] + ebase + Nt],
                            op=mybir.AluOpType.mult)
                        pt[t].append(p)
                    if prev is not None:
                        av(prev)
                        del pt[prev]
                        if i % 2 == 0:
                            fill(1)
                    prev = t
                av(prev)

                def norm():
                    for side in range(2):
                        recip = att_tmp.tile([1, 456], F16, tag="recip",
                                             name="recip")
                        with nc.allow_low_precision(
                                reason="f16 softmax reciprocal, tol 2e-2"):
                            nc.vector.reciprocal(
                                recip[0:1, 0:qN],
                                psO[side][64:65, ocol:ocol + qN])
                        rb = att_tmp.tile([64, 456], F16, tag="rb",
                                          name="rb")
                        nc.gpsimd.partition_broadcast(rb[0:64, 0:qN],
                                                      recip[0:1, 0:qN])
                        r0 = side * 64
                        nc.vector.tensor_tensor(
                            out=oT[jp][r0:r0 + 64,
                                       b * NTOK + qstart:b * NTOK + qend],
                            in0=psO[side][0:64, ocol:ocol + qN],
                            in1=rb[0:64, 0:qN],
                            op=mybir.AluOpType.mult)
                return norm

            def attn_qb0(jp, fill):
                """Merged qb0 (q rows 0:128, k-tile 0) for BOTH batches:
                one exp / recip / broadcast / norm-mult instruction pair
                instead of two."""
                psS = [ps_sT.tile([128, 456], F32, tag="psS", name="psS")
                       for _ in range(2)]
                psO = [ps_OT.tile([65, 456], F32, tag="psOT", name="psOT")
                       for _ in range(2)]
                for side in range(2):
                    r0 = side * 64
                    for b in range(2):
                        nc.tensor.matmul(
                            psS[side][0:128, b * 128:(b + 1) * 128],
                            kT[jp][r0:r0 + 64, b * NTOK:b * NTOK + 128],
                            qT[jp][r0:r0 + 64, b * NTOK:b * NTOK + 128],
                            start=True, stop=True, tile_position=(r0, 0))
                pt = []
                for side in range(2):
                    h = 2 * jp + side
                    es = es_pool.tile([128, 456], F16, tag="es", name="es")
                    nc.scalar.activation(
                        es[0:128, 0:256], psS[side][0:128, 0:256],
                        mybir.ActivationFunctionType.Exp)
                    p = p_pool.tile([128, 456], F16, tag="p", name="p")
                    for b in range(2):
                        nc.vector.tensor_tensor(
                            out=p[0:128, b * 128:(b + 1) * 128],
                            in0=es[0:128, b * 128:(b + 1) * 128],
                            in1=expb[0][0:128, h * WID[0]:h * WID[0] + 128],
                            op=mybir.AluOpType.mult)
                    pt.append(p)
                fill(1)
                for side in range(2):
                    h = 2 * jp + side
                    for b in range(2):
                        nc.tensor.matmul(
                            psO[side][0:65, b * 128:(b + 1) * 128],
                            vt[b][0][0:128, h * 65:(h + 1) * 65],
                            pt[side][0:128, b * 128:(b + 1) * 128],
                            start=True, stop=True)

                def norm():
                    for side in range(2):
                        recip = att_tmp.tile([1, 456], F16, tag="recip",
                                             name="recip")
                        with nc.allow_low_precision(
                                reason="f16 softmax reciprocal, tol 2e-2"):
                            nc.vector.reciprocal(
                                recip[0:1, 0:256], psO[side][64:65, 0:256])
                        rb = att_tmp.tile([64, 456], F16, tag="rb",
                                          name="rb")
                        nc.gpsimd.partition_broadcast(rb[0:64, 0:256],
                                                      recip[0:1, 0:256])
                        r0 = side * 64
                        o3 = oT[jp][r0:r0 + 64, 0:2 * NTOK].rearrange(
                            "p (b t) -> p b t", b=2)[:, :, 0:128]
                        nc.vector.tensor_tensor(
                            out=o3,
                            in0=psO[side][0:64, 0:256].rearrange(
                                "p (b t) -> p b t", b=2),
                            in1=rb[0:64, 0:256].rearrange(
                                "p (b t) -> p b t", b=2),
                            op=mybir.AluOpType.mult)
                return norm

            def d_phase(b, fillers):
                """D for all head-pairs of batch b; fillers[jp] is a list
                of PE filler units to sprinkle into head-pair jp."""
                for jp in range(6):
                    units = list(fillers[jp])

                    def fill(n):
                        for _ in range(n):
                            if units:
                                units.pop(0)()

                    if b == 0:
                        nqb0 = attn_qb0(jp, fill)
                        fill(1)
                        nqb0()
                    psO = [ps_OT.tile([65, 456], F32, tag="psOT",
                                      name="psOT") for _ in range(2)]
                    nrm = attn(jp, b, QBLOCKS[1][0], QBLOCKS[1][1],
                               psO, 0, fill)
                    fill(2)
                    nrm()
                    fill(len(units))

            # phase 1: B(0), C(b0), then D(b0) filled with B(jp+1) + C(b1)
            with tc.tile_pool(name="ps_qk", bufs=2, space="PSUM") as ps_qk:
                ps_v = ps_qk
                for u in b_units(0, ps_qk):
                    u()
                for u in c_units(0, ps_v, nc.vector):
                    u()
                cu1 = list(c_units(1, ps_v, nc.vector))
                cu1[0]()   # vt[1][0] needed by qb0(b1) at jp=0
                fillers0 = []
                for jp in range(6):
                    f = list(b_units(jp + 1, ps_qk)) if jp < 5 else []
                    if 1 <= jp <= 4:
                        f.append(cu1[jp])
                    fillers0.append(f)
                d_phase(0, fillers0)

            # phase 2: D(b1) filled with E(b0), then E(b1)
            with tc.tile_pool(name="ps_o", bufs=2, space="PSUM") as ps_o:
                eu0 = list(e_units(0, ps_o, yt_pool))
                fillers1 = [eu0[2 * i:2 * i + 2] for i in range(6)]
                d_phase(1, fillers1)

                # ---- E(b1) ----
                for u in e_units(1, ps_o, yt_pool):
                    u()


def kernel(x, qkv_w, qkv_b, pos_emb, out_w, out_b, rel_index):
    x = np.asarray(x, dtype=np.float32)
    qkv_w = np.asarray(qkv_w, dtype=np.float32)
    qkv_b = np.asarray(qkv_b, dtype=np.float32)
    pos_emb = np.asarray(pos_emb, dtype=np.float32)
    out_w = np.asarray(out_w, dtype=np.float32)
    out_b = np.asarray(out_b, dtype=np.float32)
    ri = np.asarray(rel_index, dtype=np.int32)

    if "nc" not in _CACHE:
        _CACHE["nc"] = _build()
    nc = _CACHE["nc"]

    exb, hqk, hv, hwo, hbc, hbrow, hocol, hx = _host_prep(
        x, qkv_w, qkv_b, pos_emb, out_w, out_b, ri)
    in_maps = []
    for c in range(NCORES):
        in_maps.append({
            "hx": hx[c], "hqk": hqk, "hv": hv, "hwo": hwo,
            "hbc": hbc, "hbrow": hbrow, "hocol": hocol, "exb": exb,
        })
    res = run_bass_kernel_spmd(nc, in_maps, core_ids=list(range(NCORES)))
    out = np.empty((B, NTOK, CDIM), dtype=np.float32)
    for c in range(NCORES):
        yt = np.asarray(res.results[c]["y"])        # [CDIM, NSEQ] f16
        yt = yt.astype(np.float32).reshape(CDIM, BLOC, NTOK)
        out[c * BLOC:(c + 1) * BLOC] = yt.transpose(1, 2, 0)
    return out



# revision 14
# speedup vs baseline: 1.0816x; 1.0816x over previous
"""Trainium2 Bass kernel for nn_EnhanceSelfAttention (B=16, N=577, C=768, H=12).

Self-contained: takes full unsharded inputs, shards batch across 8 NeuronCores
(2 batches/core), runs a fused attention kernel per core, gathers the output.

v2.1: host-side data staging removes all on-device layout work:
  - x is transposed + cast to f16 on host -> xT [128, 6*1154]
  - weights repacked per 128-row chunk so every load is one contiguous DMA
  - relative-position bias + causal mask exp()'d on host, shipped as
    per-k-tile f16 tables in the exact SBUF layout
  - softmax scale folded into the q weights/bias on host
Schedule: batch-outer attention so the other batch's v-projection and the
first batch's output projection act as PE gap-fillers inside the
ScalarE/DVE-saturated attention windows.

Per-core pipeline (f16 matmul operands, fp32 PSUM):
  B. qT,kT = Wqk^T @ xT per head-pair (interleaved into D as filler)
  C. v = x @ Wv + b per k-tile in [k, 12*65] f16 (ones col -> denominator)
  D. per (batch, head-pair): sT = kT.T@qT (two heads via PE row groups),
     p = exp(sT) * expb, OT += v.T@p over causal k-tiles; row 64 of OT is
     the denominator; divide via DVE reciprocal + GpSimd broadcast.
  E. y = OT.T @ Wout + b, per-m-tile DMAs to DRAM.
"""

import numpy as np

import concourse.bass as bass
import concourse.tile as tile
from concourse import bacc, mybir
from concourse.bass_utils import run_bass_kernel_spmd

F32 = mybir.dt.float32
F16 = mybir.dt.float16

B, NTOK, CDIM, NH, DH = 16, 577, 768, 12, 64
GRID = 24
NRD = (2 * GRID - 1) * (2 * GRID - 1) + 3  # 2212
NCORES = 8
BLOC = B // NCORES       # batches per core
NSEQ = BLOC * NTOK       # 1154
SCALE = DH ** -0.5       # 0.125
NEG = -65504.0

QBLOCKS = [(0, 128), (128, 449)]            # (qstart, qN)
KTILES = [(0, 128), (128, 128), (256, 128), (384, 128), (512, 65)]
QLO = [k0 for k0, _ in KTILES]              # per-tile stored q range [QLO[t], 577)
WID = [NTOK - q for q in QLO]               # 577, 449, 321, 193, 65
NBLK = [(0, 386), (386, 384), (770, 384)]   # token blocks for B projections
# x DMA arrives in these column blocks; jp0's B units match the finer start
XBLK = [(0, 192), (192, 194), (386, 384), (770, 384)]
NBLK0 = [(0, 192), (192, 194), (386, 384), (770, 384)]
MCH = [(0, 289), (289, 288)]                # m-chunks within a batch for E

EXB_OFF = []
_off = 0
for _t, (_k0, _pw) in enumerate(KTILES):
    EXB_OFF.append(_off)
    _off += _pw * WID[_t] * NH
EXB_TOTAL = _off

_CACHE = {}


def _check_rel_index(ri):
    """Assert the expected structure of rel_index (sanity only)."""
    assert ri.shape == (NTOK, NTOK)
    assert ri[0, 0] == NRD - 1
    assert (ri[0, 1:] == NRD - 3).all()
    assert (ri[1:, 0] == NRD - 2).all()


def _host_prep(x, qkv_w, qkv_b, pos_emb, out_w, out_b, ri):
    """Build all per-core DMA images on host."""
    _check_rel_index(ri)
    f16 = np.float16

    # exp(bias + causal mask) tables, one per k-tile, in SBUF layout
    # expb[t][p, h*W + (q - qlo)] = exp(pos_emb[h, ri[q, k0+p]] + mask(q, k0+p))
    bias = pos_emb[:, ri]                                   # [12, 577, 577]
    mask = np.where(np.tri(NTOK, dtype=np.float32) > 0, 0.0, NEG)
    exb_full = np.exp(bias + mask[None]).astype(f16)        # [12, q, k]
    exb = np.empty(EXB_TOTAL, dtype=f16)
    for t, (k0, pw) in enumerate(KTILES):
        qlo = QLO[t]
        blk = exb_full[:, qlo:, k0:k0 + pw]                 # [12, W, pw]
        exb[EXB_OFF[t]:EXB_OFF[t] + pw * WID[t] * NH] = (
            np.ascontiguousarray(blk.transpose(2, 0, 1)).reshape(-1))

    # weights: per-chunk contiguous images
    wq = (qkv_w[:, :CDIM] * SCALE).astype(f16)              # scale folded in
    wk = qkv_w[:, CDIM:2 * CDIM].astype(f16)
    wv = qkv_w[:, 2 * CDIM:].astype(f16)
    wo = out_w.astype(f16)

    def chunked(w):  # [768, 768] -> [128, 6*768]: out[p, c*768+d] = w[c*128+p, d]
        return np.ascontiguousarray(
            w.reshape(6, 128, CDIM).transpose(1, 0, 2)).reshape(128, 6 * CDIM)

    # hqk[jp] = [128, 1536]: cols 0:768 q-chunk jp, cols 768:1536 k-chunk jp
    hqk = np.empty((6, 128, 2 * CDIM), dtype=f16)
    for jp in range(6):
        for part, w in ((0, wq), (1, wk)):
            blk = w[:, jp * 128:(jp + 1) * 128]             # [768, 128]
            hqk[jp, :, part * CDIM:(part + 1) * CDIM] = (
                blk.reshape(6, 128, 128).transpose(1, 0, 2).reshape(128, CDIM))
    hqk = np.ascontiguousarray(hqk.reshape(6 * 128, 2 * CDIM))
    hv = chunked(wv)
    hwo = chunked(wo)

    # bias columns for q/k activations: [128, 12] f32
    hbc = np.empty((128, 12), dtype=np.float32)
    for r in range(12):
        if r < 6:
            hbc[:, r] = qkv_b[r * 128:(r + 1) * 128] * SCALE
        else:
            hbc[:, r] = qkv_b[CDIM + (r - 6) * 128:CDIM + (r - 5) * 128]
    # bias row for v: [1, 768] f16; out-bias as columns [128, 6] f32
    hbrow = qkv_b[2 * CDIM:].astype(f16)[None, :]
    hocol = np.ascontiguousarray(out_b.reshape(6, 128).T.astype(np.float32))

    # per-core x images: [128, 6*1154], hx[p, c*1154+t] = xshard[t, c*128+p]
    hx = []
    for c in range(NCORES):
        shard = x[c * BLOC:(c + 1) * BLOC].reshape(NSEQ, CDIM)
        hx.append(np.ascontiguousarray(
            shard.T.astype(f16).reshape(6, 128, NSEQ).transpose(1, 0, 2)
        ).reshape(128, 6 * NSEQ))
    return exb, hqk, hv, hwo, hbc, hbrow, hocol, hx


def _build():
    nc = bacc.Bacc("TRN2", target_bir_lowering=False, debug=False)

    hx_d = nc.dram_tensor("hx", [128, 6 * NSEQ], F16, kind="ExternalInput").ap()
    hqk_d = nc.dram_tensor("hqk", [6 * 128, 2 * CDIM], F16,
                           kind="ExternalInput").ap()
    hv_d = nc.dram_tensor("hv", [128, 6 * CDIM], F16, kind="ExternalInput").ap()
    hwo_d = nc.dram_tensor("hwo", [128, 6 * CDIM], F16, kind="ExternalInput").ap()
    hbc_d = nc.dram_tensor("hbc", [128, 12], F32, kind="ExternalInput").ap()
    hbrow_d = nc.dram_tensor("hbrow", [1, CDIM], F16,
                             kind="ExternalInput").ap()
    hocol_d = nc.dram_tensor("hocol", [128, 6], F32, kind="ExternalInput").ap()
    exb_d = nc.dram_tensor("exb", [EXB_TOTAL], F16, kind="ExternalInput").ap()
    # output transposed: yT[c, t]; host transposes back (free)
    y_d = nc.dram_tensor("y", [CDIM, NSEQ], F16, kind="ExternalOutput").ap()

    with tile.TileContext(nc) as tc:
        _emit(nc, tc, hx_d, hqk_d, hv_d, hwo_d, hbc_d, hbrow_d, hocol_d,
              exb_d, y_d)
    nc.compile()
    return nc


def _emit(nc, tc, hx_d, hqk_d, hv_d, hwo_d, hbc_d, hbrow_d, hocol_d,
          exb_d, y_d):
    from contextlib import ExitStack

    with ExitStack() as top:
        persist = top.enter_context(tc.tile_pool(name="persist", bufs=1))
        consts = top.enter_context(tc.tile_pool(name="consts", bufs=1))

        # ---- persistent tiles ----
        xT = consts.tile([128, 6 * NSEQ], F16, tag="xT", name="xT")
        wqk = [consts.tile([128, 2 * CDIM], F16, tag=f"wqk{j}", name=f"wqk{j}")
               for j in range(6)]
        qkb = consts.tile([128, 12], F32, tag="qkb", name="qkb")
        brow = consts.tile([1, CDIM], F16, tag="brow", name="brow")
        ocol = consts.tile([128, 6], F32, tag="ocol", name="ocol")
        ones128 = consts.tile([1, 128], F16, tag="ones128", name="ones128")
        wvall = consts.tile([128, 6 * CDIM], F16, tag="wvall", name="wvall")
        woall = consts.tile([128, 6 * CDIM], F16, tag="woall", name="woall")

        qT = [persist.tile([128, NSEQ], F16, tag=f"qT{j}", name=f"qT{j}")
              for j in range(6)]
        kT = [persist.tile([128, NSEQ], F16, tag=f"kT{j}", name=f"kT{j}")
              for j in range(6)]
        vt = [[persist.tile([128, NH * 65], F16, tag=f"v{b}_{t}", name=f"v{b}_{t}")
               for t in range(5)] for b in range(BLOC)]
        oT = [persist.tile([128, NSEQ], F16, tag=f"oT{j}", name=f"oT{j}")
              for j in range(6)]
        expb = [persist.tile([128, WID[t] * NH], F16, tag=f"expb{t}",
                             name=f"expb{t}") for t in range(5)]
        vbias = consts.tile([128, CDIM], F32, tag="vbias", name="vbias")

        # ---- DMA issue: x on sync(SP); weights on scalar(Act) queue;
        # wv + exp-bias tables on gpsimd(Pool) queue ----
        xT3 = xT[:].rearrange("p (c t) -> p c t", c=6)

        def load_exb(t):
            k0, pw = KTILES[t]
            src = bass.AP(exb_d.tensor, EXB_OFF[t],
                          [[WID[t] * NH, pw], [1, WID[t] * NH]])
            nc.gpsimd.dma_start(expb[t][0:pw, :], src)

        def load_x_cols(t0, tw):
            nc.sync.dma_start(
                xT3[:, :, t0:t0 + tw],
                bass.AP(hx_d.tensor, t0, [[6 * NSEQ, 128], [NSEQ, 6], [1, tw]]))

        nc.sync.dma_start(qkb[:], hbc_d[:, :])
        nc.sync.dma_start(brow[:], hbrow_d[:, :])
        nc.sync.dma_start(wqk[0][:], hqk_d[0:128, :])
        for t0, tw in XBLK:
            load_x_cols(t0, tw)
        for j in range(1, 6):
            nc.sync.dma_start(wqk[j][:], hqk_d[j * 128:(j + 1) * 128, :])
        nc.gpsimd.dma_start(wvall[:, 0:3 * CDIM], hv_d[:, 0:3 * CDIM])
        nc.gpsimd.dma_start(wvall[:, 3 * CDIM:], hv_d[:, 3 * CDIM:])
        load_exb(0)
        load_exb(1)
        load_exb(2)
        load_exb(3)
        load_exb(4)
        nc.gpsimd.dma_start(ocol[:], hocol_d[:, :])
        nc.gpsimd.dma_start(woall[:, 0:3 * CDIM], hwo_d[:, 0:3 * CDIM])
        nc.gpsimd.dma_start(woall[:, 3 * CDIM:], hwo_d[:, 3 * CDIM:])

        nc.vector.memset(ones128[:], 1.0)

        # PE warm-up chain (ramps the clock while DMAs land) + vbias
        # broadcast to a [128, 768] tile
        with tc.tile_pool(name="bb_psum", bufs=2, space="PSUM") as bbps:
            wps = bbps.tile([1, 456], F32, tag="warm", name="warm")
            for _ in range(10):
                nc.tensor.matmul(wps[0:1, 0:128], ones128[0:1, 0:1],
                                 ones128[0:1, :], start=True, stop=True)
            for h0 in (0, 384):
                ps = bbps.tile([128, 384], F32, tag="bb", name="bb")
                nc.tensor.matmul(ps[:], ones128[:],
                                 brow[0:1, h0:h0 + 384],
                                 start=True, stop=True)
                nc.vector.tensor_copy(vbias[:, h0:h0 + 384], ps[:])

        # ---------------- emission helpers ----------------
        def emit_b(jp, r, nb0, nbw, ps_qk):
            part = 0 if r < 6 else 1
            dst = qT[jp] if r < 6 else kT[jp]
            ps = ps_qk.tile([128, 386], F32, tag="psqk", name="psqk")
            for c in range(6):
                nc.tensor.matmul(
                    ps[0:128, 0:nbw],
                    wqk[jp][:, part * CDIM + c * 128:part * CDIM + (c + 1) * 128],
                    xT3[:, c, nb0:nb0 + nbw],
                    start=(c == 0), stop=(c == 5))
            nc.scalar.activation(
                dst[:, nb0:nb0 + nbw], ps[0:128, 0:nbw],
                mybir.ActivationFunctionType.Identity,
                bias=qkb[:, r:r + 1])

        def b_units(jp, ps_qk):
            """Emission units for head-pair jp's q/k projections (jp0 uses
            finer first blocks so PE can start as soon as x cols 0:192 land)."""
            for nb0, nbw in (NBLK0 if jp == 0 else NBLK):
                for r in (jp, jp + 6):
                    def unit(r=r, nb0=nb0, nbw=nbw):
                        emit_b(jp, r, nb0, nbw, ps_qk)
                    yield unit

        def c_units(b, ps_v, eng):
            """5 emission units (one per k-tile) of the v-projection for
            batch b. eng: engine for the PSUM->SBUF bias add."""
            for t, (k0, pw) in enumerate(KTILES):
                def unit(t=t, k0=k0, pw=pw):
                    vtile = vt[b][t]
                    for half in range(2):
                        ps = ps_v.tile([128, 386], F32, tag="psqk",
                                       name="psv")
                        for c in range(6):
                            nc.tensor.matmul(
                                ps[0:pw, 0:384],
                                xT3[:, c, b * NTOK + k0:b * NTOK + k0 + pw],
                                wvall[:, c * CDIM + half * 384:
                                      c * CDIM + (half + 1) * 384],
                                start=(c == 0), stop=(c == 5))
                        dst = vtile[0:pw, :].rearrange(
                            "p (h d) -> p h d",
                            h=NH)[:, half * 6:(half + 1) * 6, 0:64]
                        src = ps[0:pw, 0:384].rearrange("p (h d) -> p h d",
                                                        d=64)
                        bsl = vbias[0:pw, half * 384:(half + 1) * 384].rearrange(
                            "p (h d) -> p h d", d=64)
                        eng.tensor_tensor(out=dst, in0=src, in1=bsl,
                                          op=mybir.AluOpType.add)
                    nc.vector.memset(
                        vtile[0:pw, :].rearrange("p (h d) -> p h d",
                                                 h=NH)[:, :, 64:65], 1.0)
                yield unit

        def e_units(b, ps_o, yt_pool):
            """12 emission units (co-chunk j x m-chunk) of the transposed
            output projection yT[j*128:(j+1)*128, b*577+m0:m0+mw] for batch
            b. out-bias is a per-partition column folded into the ScalarE
            PSUM->SBUF copy; y ships f16 and the host transposes."""
            for j in range(6):
                for mi, (m0, mw) in enumerate(MCH):
                    def unit(j=j, mi=mi, m0=m0, mw=mw):
                        ps = ps_o.tile([128, 289], F32, tag="pso", name="pso")
                        for c in range(6):
                            nc.tensor.matmul(
                                ps[0:128, 0:mw],
                                woall[:, c * CDIM + j * 128:
                                      c * CDIM + (j + 1) * 128],
                                oT[c][:, b * NTOK + m0:b * NTOK + m0 + mw],
                                start=(c == 0), stop=(c == 5))
                        yt = yt_pool.tile([128, 289], F16, tag="yt",
                                          name="yt")
                        nc.scalar.activation(
                            yt[0:128, 0:mw], ps[0:128, 0:mw],
                            mybir.ActivationFunctionType.Identity,
                            bias=ocol[:, j:j + 1])
                        q = nc.gpsimd if (j + mi) % 2 else nc.sync
                        q.dma_start(
                            bass.AP(y_d.tensor,
                                    j * 128 * NSEQ + b * NTOK + m0,
                                    [[NSEQ, 128], [1, mw]]),
                            yt[0:128, 0:mw])
                    yield unit

        with tc.tile_pool(name="yt_pool", bufs=4) as yt_pool, \
             tc.tile_pool(name="ps_sT", bufs=4, space="PSUM") as ps_sT, \
             tc.tile_pool(name="ps_OT", bufs=2, space="PSUM") as ps_OT, \
             tc.tile_pool(name="att_tmp", bufs=4) as att_tmp, \
             tc.tile_pool(name="es_pool", bufs=7) as es_pool, \
             tc.tile_pool(name="p_pool", bufs=7) as p_pool:

            def attn(jp, b, qstart, qN, psO, ocol, fill,
                     pool_mult=False):
                """Emit S/exp/mult/AV pipeline for one (jp, b, q-block).
                Returns the deferred normalization closure."""
                qend = qstart + qN
                valid_t = [t for t in range(5) if QLO[t] < qend]
                tlast = valid_t[-1]
                prev = None

                pt = {}

                def av(t):
                    k0, pw = KTILES[t]
                    qlo = max(qstart, QLO[t])
                    off = ocol + qlo - qstart
                    Nt = qend - qlo
                    for side in range(2):
                        h = 2 * jp + side
                        nc.tensor.matmul(
                            psO[side][0:65, off:off + Nt],
                            vt[b][t][0:pw, h * 65:(h + 1) * 65],
                            pt[t][side][0:pw, 0:Nt],
                            start=(t == valid_t[0]),
                            stop=(t == tlast))

                for i, t in enumerate(valid_t):
                    k0, pw = KTILES[t]
                    qlo = max(qstart, QLO[t])
                    Nt = qend - qlo
                    ebase = qlo - QLO[t]
                    psS = [ps_sT.tile([128, 456], F32, tag="psS",
                                      name="psS") for _ in range(2)]
                    for side in range(2):
                        r0 = side * 64
                        nc.tensor.matmul(
                            psS[side][0:pw, 0:Nt],
                            kT[jp][r0:r0 + 64,
                                   b * NTOK + k0:b * NTOK + k0 + pw],
                            qT[jp][r0:r0 + 64,
                                   b * NTOK + qlo:b * NTOK + qlo + Nt],
                            start=True, stop=True,
                            tile_position=(r0, 0))
                    pt[t] = []
                    for side in range(2):
                        h = 2 * jp + side
                        es = es_pool.tile([128, 456], F16, tag="es",
                                          name="es")
                        nc.scalar.activation(
                            es[0:pw, 0:Nt], psS[side][0:pw, 0:Nt],
                            mybir.ActivationFunctionType.Exp)
                        p = p_pool.tile([128, 456], F16, tag="p", name="p")
                        meng = nc.gpsimd if (pool_mult and side == 1) \
                            else nc.vector
                        meng.tensor_tensor(
                            out=p[0:pw, 0:Nt],
                            in0=es[0:pw, 0:Nt],
                            in1=expb[t][0:pw,
                                        h * WID[t] + ebase:
                                        h * WID[t] + ebase + Nt],
                            op=mybir.AluOpType.mult)
                        pt[t].append(p)
                    if prev is not None:
                        av(prev)
                        del pt[prev]
                        if i % 2 == 0:
                            fill(1)
                    prev = t
                av(prev)

                def norm():
                    for side in range(2):
                        recip = att_tmp.tile([1, 456], F16, tag="recip",
                                             name="recip")
                        with nc.allow_low_precision(
                                reason="f16 softmax reciprocal, tol 2e-2"):
                            nc.vector.reciprocal(
                                recip[0:1, 0:qN],
                                psO[side][64:65, ocol:ocol + qN])
                        rb = att_tmp.tile([64, 456], F16, tag="rb",
                                          name="rb")
                        nc.gpsimd.partition_broadcast(rb[0:64, 0:qN],
                                                      recip[0:1, 0:qN])
                        r0 = side * 64
                        nc.vector.tensor_tensor(
                            out=oT[jp][r0:r0 + 64,
                                       b * NTOK + qstart:b * NTOK + qend],
                            in0=psO[side][0:64, ocol:ocol + qN],
                            in1=rb[0:64, 0:qN],
                            op=mybir.AluOpType.mult)
                return norm

            def attn_qb0(jp, fill):
                """Merged qb0 (q rows 0:128, k-tile 0) for BOTH batches:
                one exp / recip / broadcast / norm-mult instruction pair
                instead of two."""
                psS = [ps_sT.tile([128, 456], F32, tag="psS", name="psS")
                       for _ in range(2)]
                psO = [ps_OT.tile([65, 456], F32, tag="psOT", name="psOT")
                       for _ in range(2)]
                for side in range(2):
                    r0 = side * 64
                    for b in range(2):
                        nc.tensor.matmul(
                            psS[side][0:128, b * 128:(b + 1) * 128],
                            kT[jp][r0:r0 + 64, b * NTOK:b * NTOK + 128],
                            qT[jp][r0:r0 + 64, b * NTOK:b * NTOK + 128],
                            start=True, stop=True, tile_position=(r0, 0))
                pt = []
                for side in range(2):
                    h = 2 * jp + side
                    es = es_pool.tile([128, 456], F16, tag="es", name="es")
                    nc.scalar.activation(
                        es[0:128, 0:256], psS[side][0:128, 0:256],
                        mybir.ActivationFunctionType.Exp)
                    p = p_pool.tile([128, 456], F16, tag="p", name="p")
                    for b in range(2):
                        nc.vector.tensor_tensor(
                            out=p[0:128, b * 128:(b + 1) * 128],
                            in0=es[0:128, b * 128:(b + 1) * 128],
                            in1=expb[0][0:128, h * WID[0]:h * WID[0] + 128],
                            op=mybir.AluOpType.mult)
                    pt.append(p)
                fill(1)
                for side in range(2):
                    h = 2 * jp + side
                    for b in range(2):
                        nc.tensor.matmul(
                            psO[side][0:65, b * 128:(b + 1) * 128],
                            vt[b][0][0:128, h * 65:(h + 1) * 65],
                            pt[side][0:128, b * 128:(b + 1) * 128],
                            start=True, stop=True)

                def norm():
                    for side in range(2):
                        recip = att_tmp.tile([1, 456], F16, tag="recip",
                                             name="recip")
                        with nc.allow_low_precision(
                                reason="f16 softmax reciprocal, tol 2e-2"):
                            nc.vector.reciprocal(
                                recip[0:1, 0:256], psO[side][64:65, 0:256])
                        rb = att_tmp.tile([64, 456], F16, tag="rb",
                                          name="rb")
                        nc.gpsimd.partition_broadcast(rb[0:64, 0:256],
                                                      recip[0:1, 0:256])
                        r0 = side * 64
                        o3 = oT[jp][r0:r0 + 64, 0:2 * NTOK].rearrange(
                            "p (b t) -> p b t", b=2)[:, :, 0:128]
                        nc.vector.tensor_tensor(
                            out=o3,
                            in0=psO[side][0:64, 0:256].rearrange(
                                "p (b t) -> p b t", b=2),
                            in1=rb[0:64, 0:256].rearrange(
                                "p (b t) -> p b t", b=2),
                            op=mybir.AluOpType.mult)
                return norm

            def d_phase(b, fillers):
                """D for all head-pairs of batch b; fillers[jp] is a list
                of PE filler units to sprinkle into head-pair jp."""
                for jp in range(6):
                    units = list(fillers[jp])

                    def fill(n):
                        for _ in range(n):
                            if units:
                                units.pop(0)()

                    if b == 0:
                        nqb0 = attn_qb0(jp, fill)
                        fill(1)
                        nqb0()
                    psO = [ps_OT.tile([65, 456], F32, tag="psOT",
                                      name="psOT") for _ in range(2)]
                    nrm = attn(jp, b, QBLOCKS[1][0], QBLOCKS[1][1],
                               psO, 0, fill)
                    fill(2)
                    nrm()
                    fill(len(units))

            # phase 1: B(0), C(b0), then D(b0) filled with B(jp+1) + C(b1)
            with tc.tile_pool(name="ps_qk", bufs=2, space="PSUM") as ps_qk:
                ps_v = ps_qk
                for u in b_units(0, ps_qk):
                    u()
                for u in c_units(0, ps_v, nc.vector):
                    u()
                cu1 = list(c_units(1, ps_v, nc.vector))
                cu1[0]()   # vt[1][0] needed by qb0(b1) at jp=0
                fillers0 = []
                for jp in range(6):
                    f = list(b_units(jp + 1, ps_qk)) if jp < 5 else []
                    if 1 <= jp <= 4:
                        f.append(cu1[jp])
                    fillers0.append(f)
                d_phase(0, fillers0)

            # phase 2: D(b1) filled with E(b0), then E(b1)
            with tc.tile_pool(name="ps_o", bufs=2, space="PSUM") as ps_o:
                eu0 = list(e_units(0, ps_o, yt_pool))
                fillers1 = [eu0[2 * i:2 * i + 2] for i in range(6)]
                d_phase(1, fillers1)

                # ---- E(b1) ----
                for u in e_units(1, ps_o, yt_pool):
                    u()


def kernel(x, qkv_w, qkv_b, pos_emb, out_w, out_b, rel_index):
    x = np.asarray(x, dtype=np.float32)
    qkv_w = np.asarray(qkv_w, dtype=np.float32)
    qkv_b = np.asarray(qkv_b, dtype=np.float32)
    pos_emb = np.asarray(pos_emb, dtype=np.float32)
    out_w = np.asarray(out_w, dtype=np.float32)
    out_b = np.asarray(out_b, dtype=np.float32)
    ri = np.asarray(rel_index, dtype=np.int32)

    if "nc" not in _CACHE:
        _CACHE["nc"] = _build()
    nc = _CACHE["nc"]

    exb, hqk, hv, hwo, hbc, hbrow, hocol, hx = _host_prep(
        x, qkv_w, qkv_b, pos_emb, out_w, out_b, ri)
    in_maps = []
    for c in range(NCORES):
        in_maps.append({
            "hx": hx[c], "hqk": hqk, "hv": hv, "hwo": hwo,
            "hbc": hbc, "hbrow": hbrow, "hocol": hocol, "exb": exb,
        })
    res = run_bass_kernel_spmd(nc, in_maps, core_ids=list(range(NCORES)))
    out = np.empty((B, NTOK, CDIM), dtype=np.float32)
    for c in range(NCORES):
        yt = np.asarray(res.results[c]["y"])        # [CDIM, NSEQ] f16
        yt = yt.astype(np.float32).reshape(CDIM, BLOC, NTOK)
        out[c * BLOC:(c + 1) * BLOC] = yt.transpose(1, 2, 0)
    return out



# revision 15
# speedup vs baseline: 1.1528x; 1.0659x over previous
"""Trainium2 Bass kernel for nn_EnhanceSelfAttention (B=16, N=577, C=768, H=12).

Self-contained: takes full unsharded inputs, shards batch across 8 NeuronCores
(2 batches/core), runs a fused attention kernel per core, gathers the output.

v2.1: host-side data staging removes all on-device layout work:
  - x is transposed + cast to f16 on host -> xT [128, 6*1154]
  - weights repacked per 128-row chunk so every load is one contiguous DMA
  - relative-position bias + causal mask exp()'d on host, shipped as
    per-k-tile f16 tables in the exact SBUF layout
  - softmax scale folded into the q weights/bias on host
Schedule: batch-outer attention so the other batch's v-projection and the
first batch's output projection act as PE gap-fillers inside the
ScalarE/DVE-saturated attention windows.

Per-core pipeline (f16 matmul operands, fp32 PSUM):
  B. qT,kT = Wqk^T @ xT per head-pair (interleaved into D as filler)
  C. v = x @ Wv + b per k-tile in [k, 12*65] f16 (ones col -> denominator)
  D. per (batch, head-pair): sT = kT.T@qT (two heads via PE row groups),
     p = exp(sT) * expb, OT += v.T@p over causal k-tiles; row 64 of OT is
     the denominator; divide via DVE reciprocal + GpSimd broadcast.
  E. y = OT.T @ Wout + b, per-m-tile DMAs to DRAM.
"""

import numpy as np

import concourse.bass as bass
import concourse.tile as tile
from concourse import bacc, mybir
from concourse.bass_utils import run_bass_kernel_spmd

F32 = mybir.dt.float32
F16 = mybir.dt.float16

B, NTOK, CDIM, NH, DH = 16, 577, 768, 12, 64
GRID = 24
NRD = (2 * GRID - 1) * (2 * GRID - 1) + 3  # 2212
NCORES = 8
BLOC = B // NCORES       # batches per core
NSEQ = BLOC * NTOK       # 1154
SCALE = DH ** -0.5       # 0.125
NEG = -65504.0

QBLOCKS = [(0, 128), (128, 449)]            # (qstart, qN)
KTILES = [(0, 128), (128, 128), (256, 128), (384, 128), (512, 65)]
QLO = [k0 for k0, _ in KTILES]              # per-tile stored q range [QLO[t], 577)
WID = [NTOK - q for q in QLO]               # 577, 449, 321, 193, 65
NBLK = [(0, 386), (386, 384), (770, 384)]   # token blocks for B projections
# x DMA arrives in these column blocks; jp0's B units match the finer start
XBLK = [(0, 192), (192, 194), (386, 384), (770, 384)]
NBLK0 = [(0, 192), (192, 194), (386, 384), (770, 384)]
MCH = [(0, 289), (289, 288)]                # m-chunks within a batch for E

EXB_OFF = []
_off = 0
for _t, (_k0, _pw) in enumerate(KTILES):
    EXB_OFF.append(_off)
    _off += _pw * WID[_t] * NH
EXB_TOTAL = _off

_CACHE = {}


def _check_rel_index(ri):
    """Assert the expected structure of rel_index (sanity only)."""
    assert ri.shape == (NTOK, NTOK)
    assert ri[0, 0] == NRD - 1
    assert (ri[0, 1:] == NRD - 3).all()
    assert (ri[1:, 0] == NRD - 2).all()


def _host_prep(x, qkv_w, qkv_b, pos_emb, out_w, out_b, ri):
    """Build all per-core DMA images on host."""
    _check_rel_index(ri)
    f16 = np.float16

    # exp(bias + causal mask) tables, one per k-tile, in SBUF layout
    # expb[t][p, h*W + (q - qlo)] = exp(pos_emb[h, ri[q, k0+p]] + mask(q, k0+p))
    bias = pos_emb[:, ri]                                   # [12, 577, 577]
    mask = np.where(np.tri(NTOK, dtype=np.float32) > 0, 0.0, NEG)
    exb_full = np.exp(bias + mask[None]).astype(f16)        # [12, q, k]
    exb = np.empty(EXB_TOTAL, dtype=f16)
    for t, (k0, pw) in enumerate(KTILES):
        qlo = QLO[t]
        blk = exb_full[:, qlo:, k0:k0 + pw]                 # [12, W, pw]
        exb[EXB_OFF[t]:EXB_OFF[t] + pw * WID[t] * NH] = (
            np.ascontiguousarray(blk.transpose(2, 0, 1)).reshape(-1))

    # weights: per-chunk contiguous images
    wq = (qkv_w[:, :CDIM] * SCALE).astype(f16)              # scale folded in
    wk = qkv_w[:, CDIM:2 * CDIM].astype(f16)
    wv = qkv_w[:, 2 * CDIM:].astype(f16)
    wo = out_w.astype(f16)

    def chunked(w):  # [768, 768] -> [128, 6*768]: out[p, c*768+d] = w[c*128+p, d]
        return np.ascontiguousarray(
            w.reshape(6, 128, CDIM).transpose(1, 0, 2)).reshape(128, 6 * CDIM)

    # hqk[jp] = [128, 1536]: cols 0:768 q-chunk jp, cols 768:1536 k-chunk jp
    hqk = np.empty((6, 128, 2 * CDIM), dtype=f16)
    for jp in range(6):
        for part, w in ((0, wq), (1, wk)):
            blk = w[:, jp * 128:(jp + 1) * 128]             # [768, 128]
            hqk[jp, :, part * CDIM:(part + 1) * CDIM] = (
                blk.reshape(6, 128, 128).transpose(1, 0, 2).reshape(128, CDIM))
    hqk = np.ascontiguousarray(hqk.reshape(6 * 128, 2 * CDIM))
    hv = chunked(wv)
    hwo = chunked(wo)

    # bias columns for q/k activations: [128, 12] f32
    hbc = np.empty((128, 12), dtype=np.float32)
    for r in range(12):
        if r < 6:
            hbc[:, r] = qkv_b[r * 128:(r + 1) * 128] * SCALE
        else:
            hbc[:, r] = qkv_b[CDIM + (r - 6) * 128:CDIM + (r - 5) * 128]
    # bias row for v: [1, 768] f16; out-bias as columns [128, 6] f32
    hbrow = qkv_b[2 * CDIM:].astype(f16)[None, :]
    hocol = np.ascontiguousarray(out_b.reshape(6, 128).T.astype(np.float32))

    # per-core x images: [128, 6*1154], hx[p, c*1154+t] = xshard[t, c*128+p]
    hx = []
    for c in range(NCORES):
        shard = x[c * BLOC:(c + 1) * BLOC].reshape(NSEQ, CDIM)
        hx.append(np.ascontiguousarray(
            shard.T.astype(f16).reshape(6, 128, NSEQ).transpose(1, 0, 2)
        ).reshape(128, 6 * NSEQ))
    return exb, hqk, hv, hwo, hbc, hbrow, hocol, hx


def _build():
    nc = bacc.Bacc("TRN2", target_bir_lowering=False, debug=False)

    hx_d = nc.dram_tensor("hx", [128, 6 * NSEQ], F16, kind="ExternalInput").ap()
    hqk_d = nc.dram_tensor("hqk", [6 * 128, 2 * CDIM], F16,
                           kind="ExternalInput").ap()
    hv_d = nc.dram_tensor("hv", [128, 6 * CDIM], F16, kind="ExternalInput").ap()
    hwo_d = nc.dram_tensor("hwo", [128, 6 * CDIM], F16, kind="ExternalInput").ap()
    hbc_d = nc.dram_tensor("hbc", [128, 12], F32, kind="ExternalInput").ap()
    hbrow_d = nc.dram_tensor("hbrow", [1, CDIM], F16,
                             kind="ExternalInput").ap()
    hocol_d = nc.dram_tensor("hocol", [128, 6], F32, kind="ExternalInput").ap()
    exb_d = nc.dram_tensor("exb", [EXB_TOTAL], F16, kind="ExternalInput").ap()
    # output transposed: yT[c, t]; host transposes back (free)
    y_d = nc.dram_tensor("y", [CDIM, NSEQ], F16, kind="ExternalOutput").ap()

    with tile.TileContext(nc) as tc:
        _emit(nc, tc, hx_d, hqk_d, hv_d, hwo_d, hbc_d, hbrow_d, hocol_d,
              exb_d, y_d)
    nc.compile()
    return nc


def _emit(nc, tc, hx_d, hqk_d, hv_d, hwo_d, hbc_d, hbrow_d, hocol_d,
          exb_d, y_d):
    from contextlib import ExitStack

    with ExitStack() as top:
        persist = top.enter_context(tc.tile_pool(name="persist", bufs=1))
        consts = top.enter_context(tc.tile_pool(name="consts", bufs=1))

        # ---- persistent tiles ----
        xT = consts.tile([128, 6 * NSEQ], F16, tag="xT", name="xT")
        wqk = [consts.tile([128, 2 * CDIM], F16, tag=f"wqk{j}", name=f"wqk{j}")
               for j in range(6)]
        qkb = consts.tile([128, 12], F32, tag="qkb", name="qkb")
        brow = consts.tile([1, CDIM], F16, tag="brow", name="brow")
        ocol = consts.tile([128, 6], F32, tag="ocol", name="ocol")
        ones128 = consts.tile([1, 128], F16, tag="ones128", name="ones128")
        wvall = consts.tile([128, 6 * CDIM], F16, tag="wvall", name="wvall")
        woall = consts.tile([128, 6 * CDIM], F16, tag="woall", name="woall")

        qT = [persist.tile([128, NSEQ], F16, tag=f"qT{j}", name=f"qT{j}")
              for j in range(6)]
        kT = [persist.tile([128, NSEQ], F16, tag=f"kT{j}", name=f"kT{j}")
              for j in range(6)]
        vt = [[persist.tile([128, NH * 65], F16, tag=f"v{b}_{t}", name=f"v{b}_{t}")
               for t in range(5)] for b in range(BLOC)]
        oT = [persist.tile([128, NSEQ], F16, tag=f"oT{j}", name=f"oT{j}")
              for j in range(6)]
        expb = [persist.tile([128, WID[t] * NH], F16, tag=f"expb{t}",
                             name=f"expb{t}") for t in range(5)]
        vbias = consts.tile([128, CDIM], F32, tag="vbias", name="vbias")

        # ---- DMA issue: x on sync(SP); weights on scalar(Act) queue;
        # wv + exp-bias tables on gpsimd(Pool) queue ----
        xT3 = xT[:].rearrange("p (c t) -> p c t", c=6)

        def load_exb(t):
            k0, pw = KTILES[t]
            src = bass.AP(exb_d.tensor, EXB_OFF[t],
                          [[WID[t] * NH, pw], [1, WID[t] * NH]])
            nc.gpsimd.dma_start(expb[t][0:pw, :], src)

        def load_x_cols(t0, tw):
            nc.sync.dma_start(
                xT3[:, :, t0:t0 + tw],
                bass.AP(hx_d.tensor, t0, [[6 * NSEQ, 128], [NSEQ, 6], [1, tw]]))

        def load_x_cols_q(q, t0, tw):
            q.dma_start(
                xT3[:, :, t0:t0 + tw],
                bass.AP(hx_d.tensor, t0, [[6 * NSEQ, 128], [NSEQ, 6], [1, tw]]))

        # sync(SP) queue: bias cols, first qk weights, x even blocks, rest of
        # qk weights.  pool queue (cheap 25ns issue): x odd blocks, v/out
        # weights, exp-bias tables.  The two queues' transfers overlap.
        nc.sync.dma_start(qkb[:], hbc_d[:, :])
        nc.sync.dma_start(brow[:], hbrow_d[:, :])
        nc.sync.dma_start(wqk[0][:], hqk_d[0:128, :])
        load_x_cols_q(nc.sync, *XBLK[0])
        load_x_cols_q(nc.gpsimd, *XBLK[1])
        load_x_cols_q(nc.sync, *XBLK[2])
        load_x_cols_q(nc.gpsimd, *XBLK[3])
        for j in range(1, 6):
            nc.sync.dma_start(wqk[j][:], hqk_d[j * 128:(j + 1) * 128, :])
        nc.gpsimd.dma_start(wvall[:, 0:3 * CDIM], hv_d[:, 0:3 * CDIM])
        nc.gpsimd.dma_start(wvall[:, 3 * CDIM:], hv_d[:, 3 * CDIM:])
        load_exb(0)
        load_exb(1)
        load_exb(2)
        load_exb(3)
        load_exb(4)
        nc.gpsimd.dma_start(ocol[:], hocol_d[:, :])
        nc.gpsimd.dma_start(woall[:, 0:3 * CDIM], hwo_d[:, 0:3 * CDIM])
        nc.gpsimd.dma_start(woall[:, 3 * CDIM:], hwo_d[:, 3 * CDIM:])

        nc.vector.memset(ones128[:], 1.0)

        # PE warm-up chain (ramps the clock while DMAs land) + vbias
        # broadcast to a [128, 768] tile
        with tc.tile_pool(name="bb_psum", bufs=2, space="PSUM") as bbps:
            wps = bbps.tile([1, 456], F32, tag="warm", name="warm")
            for _ in range(10):
                nc.tensor.matmul(wps[0:1, 0:128], ones128[0:1, 0:1],
                                 ones128[0:1, :], start=True, stop=True)
            for h0 in (0, 384):
                ps = bbps.tile([128, 384], F32, tag="bb", name="bb")
                nc.tensor.matmul(ps[:], ones128[:],
                                 brow[0:1, h0:h0 + 384],
                                 start=True, stop=True)
                nc.vector.tensor_copy(vbias[:, h0:h0 + 384], ps[:])

        # ---------------- emission helpers ----------------
        def emit_b(jp, r, nb0, nbw, ps_qk):
            part = 0 if r < 6 else 1
            dst = qT[jp] if r < 6 else kT[jp]
            ps = ps_qk.tile([128, 386], F32, tag="psqk", name="psqk")
            for c in range(6):
                nc.tensor.matmul(
                    ps[0:128, 0:nbw],
                    wqk[jp][:, part * CDIM + c * 128:part * CDIM + (c + 1) * 128],
                    xT3[:, c, nb0:nb0 + nbw],
                    start=(c == 0), stop=(c == 5))
            nc.scalar.activation(
                dst[:, nb0:nb0 + nbw], ps[0:128, 0:nbw],
                mybir.ActivationFunctionType.Identity,
                bias=qkb[:, r:r + 1])

        def b_units(jp, ps_qk):
            """Emission units for head-pair jp's q/k projections (jp0 uses
            finer first blocks so PE can start as soon as x cols 0:192 land)."""
            for nb0, nbw in (NBLK0 if jp == 0 else NBLK):
                for r in (jp, jp + 6):
                    def unit(r=r, nb0=nb0, nbw=nbw):
                        emit_b(jp, r, nb0, nbw, ps_qk)
                    yield unit

        def c_units(b, ps_v, eng):
            """5 emission units (one per k-tile) of the v-projection for
            batch b. eng: engine for the PSUM->SBUF bias add."""
            for t, (k0, pw) in enumerate(KTILES):
                def unit(t=t, k0=k0, pw=pw):
                    vtile = vt[b][t]
                    for half in range(2):
                        ps = ps_v.tile([128, 386], F32, tag="psqk",
                                       name="psv")
                        for c in range(6):
                            nc.tensor.matmul(
                                ps[0:pw, 0:384],
                                xT3[:, c, b * NTOK + k0:b * NTOK + k0 + pw],
                                wvall[:, c * CDIM + half * 384:
                                      c * CDIM + (half + 1) * 384],
                                start=(c == 0), stop=(c == 5))
                        dst = vtile[0:pw, :].rearrange(
                            "p (h d) -> p h d",
                            h=NH)[:, half * 6:(half + 1) * 6, 0:64]
                        src = ps[0:pw, 0:384].rearrange("p (h d) -> p h d",
                                                        d=64)
                        bsl = vbias[0:pw, half * 384:(half + 1) * 384].rearrange(
                            "p (h d) -> p h d", d=64)
                        eng.tensor_tensor(out=dst, in0=src, in1=bsl,
                                          op=mybir.AluOpType.add)
                    nc.vector.memset(
                        vtile[0:pw, :].rearrange("p (h d) -> p h d",
                                                 h=NH)[:, :, 64:65], 1.0)
                yield unit

        def e_units(b, ps_o, yt_pool):
            """12 emission units (co-chunk j x m-chunk) of the transposed
            output projection yT[j*128:(j+1)*128, b*577+m0:m0+mw] for batch
            b. out-bias is a per-partition column folded into the ScalarE
            PSUM->SBUF copy; y ships f16 and the host transposes."""
            for j in range(6):
                for mi, (m0, mw) in enumerate(MCH):
                    def unit(j=j, mi=mi, m0=m0, mw=mw):
                        ps = ps_o.tile([128, 289], F32, tag="pso", name="pso")
                        for c in range(6):
                            nc.tensor.matmul(
                                ps[0:128, 0:mw],
                                woall[:, c * CDIM + j * 128:
                                      c * CDIM + (j + 1) * 128],
                                oT[c][:, b * NTOK + m0:b * NTOK + m0 + mw],
                                start=(c == 0), stop=(c == 5))
                        yt = yt_pool.tile([128, 289], F16, tag="yt",
                                          name="yt")
                        nc.scalar.activation(
                            yt[0:128, 0:mw], ps[0:128, 0:mw],
                            mybir.ActivationFunctionType.Identity,
                            bias=ocol[:, j:j + 1])
                        q = nc.gpsimd if (j + mi) % 2 else nc.sync
                        q.dma_start(
                            bass.AP(y_d.tensor,
                                    j * 128 * NSEQ + b * NTOK + m0,
                                    [[NSEQ, 128], [1, mw]]),
                            yt[0:128, 0:mw])
                    yield unit

        with tc.tile_pool(name="yt_pool", bufs=4) as yt_pool, \
             tc.tile_pool(name="ps_sT", bufs=4, space="PSUM") as ps_sT, \
             tc.tile_pool(name="ps_OT", bufs=2, space="PSUM") as ps_OT, \
             tc.tile_pool(name="att_tmp", bufs=4) as att_tmp, \
             tc.tile_pool(name="es_pool", bufs=7) as es_pool, \
             tc.tile_pool(name="p_pool", bufs=7) as p_pool:

            def attn(jp, b, qstart, qN, psO, ocol, fill,
                     pool_mult=False):
                """Emit S/exp/mult/AV pipeline for one (jp, b, q-block).
                Returns the deferred normalization closure."""
                qend = qstart + qN
                valid_t = [t for t in range(5) if QLO[t] < qend]
                tlast = valid_t[-1]
                prev = None

                pt = {}

                def av(t):
                    k0, pw = KTILES[t]
                    qlo = max(qstart, QLO[t])
                    off = ocol + qlo - qstart
                    Nt = qend - qlo
                    for side in range(2):
                        h = 2 * jp + side
                        nc.tensor.matmul(
                            psO[side][0:65, off:off + Nt],
                            vt[b][t][0:pw, h * 65:(h + 1) * 65],
                            pt[t][side][0:pw, 0:Nt],
                            start=(t == valid_t[0]),
                            stop=(t == tlast))

                for i, t in enumerate(valid_t):
                    k0, pw = KTILES[t]
                    qlo = max(qstart, QLO[t])
                    Nt = qend - qlo
                    ebase = qlo - QLO[t]
                    psS = [ps_sT.tile([128, 456], F32, tag="psS",
                                      name="psS") for _ in range(2)]
                    for side in range(2):
                        r0 = side * 64
                        nc.tensor.matmul(
                            psS[side][0:pw, 0:Nt],
                            kT[jp][r0:r0 + 64,
                                   b * NTOK + k0:b * NTOK + k0 + pw],
                            qT[jp][r0:r0 + 64,
                                   b * NTOK + qlo:b * NTOK + qlo + Nt],
                            start=True, stop=True,
                            tile_position=(r0, 0))
                    pt[t] = []
                    for side in range(2):
                        h = 2 * jp + side
                        es = es_pool.tile([128, 456], F16, tag="es",
                                          name="es")
                        nc.scalar.activation(
                            es[0:pw, 0:Nt], psS[side][0:pw, 0:Nt],
                            mybir.ActivationFunctionType.Exp)
                        p = p_pool.tile([128, 456], F16, tag="p", name="p")
                        meng = nc.gpsimd if (pool_mult and side == 1) \
                            else nc.vector
                        meng.tensor_tensor(
                            out=p[0:pw, 0:Nt],
                            in0=es[0:pw, 0:Nt],
                            in1=expb[t][0:pw,
                                        h * WID[t] + ebase:
                                        h * WID[t] + ebase + Nt],
                            op=mybir.AluOpType.mult)
                        pt[t].append(p)
                    if prev is not None:
                        av(prev)
                        del pt[prev]
                        if i % 2 == 0:
                            fill(1)
                    prev = t
                av(prev)

                def norm():
                    for side in range(2):
                        recip = att_tmp.tile([1, 456], F16, tag="recip",
                                             name="recip")
                        with nc.allow_low_precision(
                                reason="f16 softmax reciprocal, tol 2e-2"):
                            nc.vector.reciprocal(
                                recip[0:1, 0:qN],
                                psO[side][64:65, ocol:ocol + qN])
                        rb = att_tmp.tile([64, 456], F16, tag="rb",
                                          name="rb")
                        nc.gpsimd.partition_broadcast(rb[0:64, 0:qN],
                                                      recip[0:1, 0:qN])
                        r0 = side * 64
                        nc.vector.tensor_tensor(
                            out=oT[jp][r0:r0 + 64,
                                       b * NTOK + qstart:b * NTOK + qend],
                            in0=psO[side][0:64, ocol:ocol + qN],
                            in1=rb[0:64, 0:qN],
                            op=mybir.AluOpType.mult)
                return norm

            def attn_qb0(jp, fill):
                """Merged qb0 (q rows 0:128, k-tile 0) for BOTH batches:
                one exp / recip / broadcast / norm-mult instruction pair
                instead of two."""
                psS = [ps_sT.tile([128, 456], F32, tag="psS", name="psS")
                       for _ in range(2)]
                psO = [ps_OT.tile([65, 456], F32, tag="psOT", name="psOT")
                       for _ in range(2)]
                for side in range(2):
                    r0 = side * 64
                    for b in range(2):
                        nc.tensor.matmul(
                            psS[side][0:128, b * 128:(b + 1) * 128],
                            kT[jp][r0:r0 + 64, b * NTOK:b * NTOK + 128],
                            qT[jp][r0:r0 + 64, b * NTOK:b * NTOK + 128],
                            start=True, stop=True, tile_position=(r0, 0))
                pt = []
                for side in range(2):
                    h = 2 * jp + side
                    es = es_pool.tile([128, 456], F16, tag="es", name="es")
                    nc.scalar.activation(
                        es[0:128, 0:256], psS[side][0:128, 0:256],
                        mybir.ActivationFunctionType.Exp)
                    p = p_pool.tile([128, 456], F16, tag="p", name="p")
                    for b in range(2):
                        nc.vector.tensor_tensor(
                            out=p[0:128, b * 128:(b + 1) * 128],
                            in0=es[0:128, b * 128:(b + 1) * 128],
                            in1=expb[0][0:128, h * WID[0]:h * WID[0] + 128],
                            op=mybir.AluOpType.mult)
                    pt.append(p)
                fill(1)
                for side in range(2):
                    h = 2 * jp + side
                    for b in range(2):
                        nc.tensor.matmul(
                            psO[side][0:65, b * 128:(b + 1) * 128],
                            vt[b][0][0:128, h * 65:(h + 1) * 65],
                            pt[side][0:128, b * 128:(b + 1) * 128],
                            start=True, stop=True)

                def norm():
                    for side in range(2):
                        recip = att_tmp.tile([1, 456], F16, tag="recip",
                                             name="recip")
                        with nc.allow_low_precision(
                                reason="f16 softmax reciprocal, tol 2e-2"):
                            nc.vector.reciprocal(
                                recip[0:1, 0:256], psO[side][64:65, 0:256])
                        rb = att_tmp.tile([64, 456], F16, tag="rb",
                                          name="rb")
                        nc.gpsimd.partition_broadcast(rb[0:64, 0:256],
                                                      recip[0:1, 0:256])
                        r0 = side * 64
                        o3 = oT[jp][r0:r0 + 64, 0:2 * NTOK].rearrange(
                            "p (b t) -> p b t", b=2)[:, :, 0:128]
                        nc.vector.tensor_tensor(
                            out=o3,
                            in0=psO[side][0:64, 0:256].rearrange(
                                "p (b t) -> p b t", b=2),
                            in1=rb[0:64, 0:256].rearrange(
                                "p (b t) -> p b t", b=2),
                            op=mybir.AluOpType.mult)
                return norm

            def d_phase(b, fillers):
                """D for all head-pairs of batch b; fillers[jp] is a list
                of PE filler units to sprinkle into head-pair jp."""
                for jp in range(6):
                    units = list(fillers[jp])

                    def fill(n):
                        for _ in range(n):
                            if units:
                                units.pop(0)()

                    if b == 0:
                        nqb0 = attn_qb0(jp, fill)
                        fill(1)
                        nqb0()
                    psO = [ps_OT.tile([65, 456], F32, tag="psOT",
                                      name="psOT") for _ in range(2)]
                    nrm = attn(jp, b, QBLOCKS[1][0], QBLOCKS[1][1],
                               psO, 0, fill)
                    fill(2)
                    nrm()
                    fill(len(units))

            # phase 1: B(0), C(b0), then D(b0) filled with B(jp+1) + C(b1)
            with tc.tile_pool(name="ps_qk", bufs=2, space="PSUM") as ps_qk:
                ps_v = ps_qk
                for u in b_units(0, ps_qk):
                    u()
                for u in c_units(0, ps_v, nc.vector):
                    u()
                cu1 = list(c_units(1, ps_v, nc.vector))
                cu1[0]()   # vt[1][0] needed by qb0(b1) at jp=0
                fillers0 = []
                for jp in range(6):
                    f = list(b_units(jp + 1, ps_qk)) if jp < 5 else []
                    if 1 <= jp <= 4:
                        f.append(cu1[jp])
                    fillers0.append(f)
                d_phase(0, fillers0)

            # phase 2: D(b1) filled with E(b0), then E(b1)
            with tc.tile_pool(name="ps_o", bufs=2, space="PSUM") as ps_o:
                eu0 = list(e_units(0, ps_o, yt_pool))
                fillers1 = [eu0[2 * i:2 * i + 2] for i in range(6)]
                d_phase(1, fillers1)

                # ---- E(b1) ----
                for u in e_units(1, ps_o, yt_pool):
                    u()


def kernel(x, qkv_w, qkv_b, pos_emb, out_w, out_b, rel_index):
    x = np.asarray(x, dtype=np.float32)
    qkv_w = np.asarray(qkv_w, dtype=np.float32)
    qkv_b = np.asarray(qkv_b, dtype=np.float32)
    pos_emb = np.asarray(pos_emb, dtype=np.float32)
    out_w = np.asarray(out_w, dtype=np.float32)
    out_b = np.asarray(out_b, dtype=np.float32)
    ri = np.asarray(rel_index, dtype=np.int32)

    if "nc" not in _CACHE:
        _CACHE["nc"] = _build()
    nc = _CACHE["nc"]

    exb, hqk, hv, hwo, hbc, hbrow, hocol, hx = _host_prep(
        x, qkv_w, qkv_b, pos_emb, out_w, out_b, ri)
    in_maps = []
    for c in range(NCORES):
        in_maps.append({
            "hx": hx[c], "hqk": hqk, "hv": hv, "hwo": hwo,
            "hbc": hbc, "hbrow": hbrow, "hocol": hocol, "exb": exb,
        })
    res = run_bass_kernel_spmd(nc, in_maps, core_ids=list(range(NCORES)))
    out = np.empty((B, NTOK, CDIM), dtype=np.float32)
    for c in range(NCORES):
        yt = np.asarray(res.results[c]["y"])        # [CDIM, NSEQ] f16
        yt = yt.astype(np.float32).reshape(CDIM, BLOC, NTOK)
        out[c * BLOC:(c + 1) * BLOC] = yt.transpose(1, 2, 0)
    return out

